# revision 3
# baseline (speedup 1.0000x reference)
"""MinGRU cell on 8 TRN2 NeuronCores.

Math (per batch b):
    g = sigmoid(x @ Wg.T + bg)          # [L, D]
    c = tanh(x @ Wh.T + bh)             # [L, D]
    h_t = g_t * h_{t-1} + (1 - g_t) * c_t   (h_0 init = hidden)

Sharding: data-parallel over batch B=8 -> one batch per core, no collectives.

Device layout: everything is kept "D on partitions, L on free dim":
  - host feeds xT = x[b].T  [D, L]  (contiguous DMA loads)
  - matmuls compute outT tiles [e_block=128, token_chunk=512] with PSUM
    accumulation over the 8 k-blocks of D
  - ScalarE applies sigmoid/tanh with the per-partition bias fused
  - VectorE computes d1 = (g-1)*c, then tensor_tensor_scan gives
    h = g*h_prev - d1 = g*h_prev + (1-g)*c along the free (token) dim
  - output is written back as outT [D, L]; host transposes to [L, D]

Matmul dtype: float32r (full-rate fp32 on the PE for N>=256).
"""

import numpy as np

import concourse.bacc as bacc
import concourse.tile as tile
import concourse.mybir as mybir
from concourse import bass_utils

B = 8
L = 4096
D = 1024
P = 128
NCH = 512          # token chunk (one fp32 PSUM bank)
KD = D // P        # 8 contraction blocks
NE = D // P        # 8 output-dim blocks
NCHUNK = L // NCH  # 8 token chunks

F32 = mybir.dt.float32
MM_DT = mybir.dt.float32r  # full-rate fp32 matmul


def build_nc():
    nc = bacc.Bacc("TRN2", target_bir_lowering=False, debug=False)

    xT = nc.dram_tensor("xT", [D, L], MM_DT, kind="ExternalInput").ap()
    WgT = nc.dram_tensor("WgT", [D, D], MM_DT, kind="ExternalInput").ap()
    WhT = nc.dram_tensor("WhT", [D, D], MM_DT, kind="ExternalInput").ap()
    bg = nc.dram_tensor("bg", [D], F32, kind="ExternalInput").ap()
    bh = nc.dram_tensor("bh", [D], F32, kind="ExternalInput").ap()
    hidden = nc.dram_tensor("hidden", [D], F32, kind="ExternalInput").ap()
    outT = nc.dram_tensor("outT", [D, L], F32, kind="ExternalOutput").ap()

    xT_r = xT.rearrange("(kd p) l -> p kd l", p=P)      # [128, 8, 4096]
    out_r = outT.rearrange("(e p) l -> p e l", p=P)     # [128, 8, 4096]
    wgT_r = WgT.rearrange("(kd p) e -> p kd e", p=P)    # [128, 8, 1024]
    whT_r = WhT.rearrange("(kd p) e -> p kd e", p=P)
    bg_r = bg.rearrange("(e p) -> p e", p=P)            # [128, 8]
    bh_r = bh.rearrange("(e p) -> p e", p=P)
    h0_r = hidden.rearrange("(e p) -> p e", p=P)

    ACT = mybir.ActivationFunctionType
    ALU = mybir.AluOpType

    with tile.TileContext(nc) as tc:
        with (
            tc.tile_pool(name="const", bufs=1) as const,
            tc.tile_pool(name="xin", bufs=2) as xpool,
            tc.tile_pool(name="gc", bufs=3) as gc,
            tc.tile_pool(name="hout", bufs=2) as hpool,
            tc.tile_pool(name="psum", bufs=2, space="PSUM") as pp,
        ):
            wg_sb = const.tile([P, KD, D], MM_DT)
            wh_sb = const.tile([P, KD, D], MM_DT)
            nc.sync.dma_start(out=wg_sb, in_=wgT_r)
            nc.sync.dma_start(out=wh_sb, in_=whT_r)
            bg_sb = const.tile([P, NE], F32)
            bh_sb = const.tile([P, NE], F32)
            h0_sb = const.tile([P, NE], F32)
            nc.sync.dma_start(out=bg_sb, in_=bg_r)
            nc.sync.dma_start(out=bh_sb, in_=bh_r)
            nc.sync.dma_start(out=h0_sb, in_=h0_r)

            prev_hout = None
            for n in range(NCHUNK):
                lsl = slice(n * NCH, (n + 1) * NCH)
                xin = xpool.tile([P, KD, NCH], MM_DT, tag="xin")
                nc.sync.dma_start(out=xin, in_=xT_r[:, :, lsl])

                hout = hpool.tile([P, NE, NCH], F32, tag="hout")
                for e in range(NE):
                    esl = slice(e * P, (e + 1) * P)
                    pg = pp.tile([P, NCH], F32, tag="pg")
                    pc = pp.tile([P, NCH], F32, tag="pc")
                    for kd in range(KD):
                        nc.tensor.matmul(
                            pg,
                            lhsT=wg_sb[:, kd, esl],
                            rhs=xin[:, kd, :],
                            start=(kd == 0),
                            stop=(kd == KD - 1),
                        )
                    for kd in range(KD):
                        nc.tensor.matmul(
                            pc,
                            lhsT=wh_sb[:, kd, esl],
                            rhs=xin[:, kd, :],
                            start=(kd == 0),
                            stop=(kd == KD - 1),
                        )
                    g = gc.tile([P, NCH], F32, tag="g")
                    nc.scalar.activation(
                        out=g, in_=pg, func=ACT.Sigmoid, bias=bg_sb[:, e : e + 1]
                    )
                    c = gc.tile([P, NCH], F32, tag="c")
                    nc.scalar.activation(
                        out=c, in_=pc, func=ACT.Tanh, bias=bh_sb[:, e : e + 1]
                    )
                    d1 = gc.tile([P, NCH], F32, tag="d1")
                    nc.vector.scalar_tensor_tensor(
                        out=d1, in0=g, scalar=1.0, in1=c,
                        op0=ALU.subtract, op1=ALU.mult,
                    )
                    init = (
                        h0_sb[:, e : e + 1]
                        if n == 0
                        else prev_hout[:, e, NCH - 1 : NCH]
                    )
                    nc.vector.tensor_tensor_scan(
                        out=hout[:, e, :], data0=g, data1=d1, initial=init,
                        op0=ALU.mult, op1=ALU.subtract,
                    )
                nc.sync.dma_start(out=out_r[:, :, lsl], in_=hout)
                prev_hout = hout

    nc.compile()
    return nc


_NC_CACHE = None


def _get_nc():
    global _NC_CACHE
    if _NC_CACHE is None:
        _NC_CACHE = build_nc()
    return _NC_CACHE


def kernel(x, hidden, Wg, bg, Wh, bh):
    x = np.ascontiguousarray(np.asarray(x, dtype=np.float32))
    hidden = np.ascontiguousarray(np.asarray(hidden, dtype=np.float32))
    Wg = np.asarray(Wg, dtype=np.float32)
    bg = np.ascontiguousarray(np.asarray(bg, dtype=np.float32))
    Wh = np.asarray(Wh, dtype=np.float32)
    bh = np.ascontiguousarray(np.asarray(bh, dtype=np.float32))

    nc = _get_nc()

    xT = np.ascontiguousarray(x.transpose(0, 2, 1))   # [B, D, L]
    WgT = np.ascontiguousarray(Wg.T)
    WhT = np.ascontiguousarray(Wh.T)

    in_maps = [
        {
            "xT": xT[b],
            "WgT": WgT,
            "WhT": WhT,
            "bg": bg,
            "bh": bh,
            "hidden": hidden[b],
        }
        for b in range(B)
    ]
    res = bass_utils.run_bass_kernel_spmd(nc, in_maps, core_ids=list(range(B)))
    out = np.stack([res.results[b]["outT"].T for b in range(B)])  # [B, L, D]
    return np.ascontiguousarray(out.astype(np.float32))


# revision 4
# speedup vs baseline: 1.0644x; 1.0644x over previous
"""MinGRU cell on 8 TRN2 NeuronCores.

Math (per batch b):
    g = sigmoid(x @ Wg.T + bg)          # [L, D]
    c = tanh(x @ Wh.T + bh)             # [L, D]
    h_t = g_t * h_{t-1} + (1 - g_t) * c_t   (h_0 init = hidden)

Sharding: data-parallel over batch B=8 -> one batch per core, no collectives.

Device layout: everything is kept "D on partitions, L on free dim":
  - host feeds xT = x[b].T  [D, L]  (contiguous DMA loads)
  - matmuls compute outT tiles [e_block=128, token_chunk=512] with PSUM
    accumulation over the 8 k-blocks of D
  - ScalarE applies sigmoid/tanh with the per-partition bias fused
  - VectorE computes d1 = (g-1)*c, then tensor_tensor_scan gives
    h = g*h_prev - d1 = g*h_prev + (1-g)*c along the free (token) dim
  - output is written back as outT [D, L]; host transposes to [L, D]

Matmul dtype: float32r (full-rate fp32 on the PE for N>=256).
"""

import numpy as np

import concourse.bacc as bacc
import concourse.tile as tile
import concourse.mybir as mybir
from concourse import bass_utils

B = 8
L = 4096
D = 1024
P = 128
NCH = 512          # token chunk (one fp32 PSUM bank)
KD = D // P        # 8 contraction blocks
NE = D // P        # 8 output-dim blocks
NCHUNK = L // NCH  # 8 token chunks

F32 = mybir.dt.float32
MM_DT = mybir.dt.float32r  # full-rate fp32 matmul


def build_nc():
    nc = bacc.Bacc("TRN2", target_bir_lowering=False, debug=False)

    xT = nc.dram_tensor("xT", [D, L], MM_DT, kind="ExternalInput").ap()
    WgT = nc.dram_tensor("WgT", [D, D], MM_DT, kind="ExternalInput").ap()
    WhT = nc.dram_tensor("WhT", [D, D], MM_DT, kind="ExternalInput").ap()
    bg = nc.dram_tensor("bg", [D], F32, kind="ExternalInput").ap()
    bh = nc.dram_tensor("bh", [D], F32, kind="ExternalInput").ap()
    hidden = nc.dram_tensor("hidden", [D], F32, kind="ExternalInput").ap()
    outT = nc.dram_tensor("outT", [D, L], F32, kind="ExternalOutput").ap()

    xT_r = xT.rearrange("(kd p) l -> p kd l", p=P)      # [128, 8, 4096]
    out_r = outT.rearrange("(e p) l -> p e l", p=P)     # [128, 8, 4096]
    wgT_r = WgT.rearrange("(kd p) e -> p kd e", p=P)    # [128, 8, 1024]
    whT_r = WhT.rearrange("(kd p) e -> p kd e", p=P)
    bg_r = bg.rearrange("(e p) -> p e", p=P)            # [128, 8]
    bh_r = bh.rearrange("(e p) -> p e", p=P)
    h0_r = hidden.rearrange("(e p) -> p e", p=P)

    ACT = mybir.ActivationFunctionType
    ALU = mybir.AluOpType

    with tile.TileContext(nc) as tc:
        with (
            tc.tile_pool(name="const", bufs=1) as const,
            tc.tile_pool(name="xin", bufs=2) as xpool,
            tc.tile_pool(name="gc", bufs=3) as gc,
            tc.tile_pool(name="hout", bufs=2) as hpool,
            tc.tile_pool(name="psum", bufs=2, space="PSUM") as pp,
        ):
            # First x chunk before the weights: the first matmul needs
            # xin(chunk0) + wg[kd=0] only, so PE can start ~3us in.
            xin0 = xpool.tile([P, KD, NCH], MM_DT, tag="xin")
            nc.sync.dma_start(out=xin0, in_=xT_r[:, :, 0:NCH])

            # Per-kd weight tiles: each 512KB DMA unblocks its kd's matmuls.
            wg_sb = []
            wh_sb = []
            for kd in range(KD):
                wgt = const.tile([P, D], MM_DT, tag=f"wg{kd}")
                nc.sync.dma_start(out=wgt, in_=wgT_r[:, kd, :])
                wht = const.tile([P, D], MM_DT, tag=f"wh{kd}")
                nc.sync.dma_start(out=wht, in_=whT_r[:, kd, :])
                wg_sb.append(wgt)
                wh_sb.append(wht)

            bg_sb = const.tile([P, NE], F32)
            bh_sb = const.tile([P, NE], F32)
            h0_sb = const.tile([P, NE], F32)
            nc.sync.dma_start(out=bg_sb, in_=bg_r)
            nc.sync.dma_start(out=bh_sb, in_=bh_r)
            nc.sync.dma_start(out=h0_sb, in_=h0_r)

            prev_h = [None] * NE
            for n in range(NCHUNK):
                lsl = slice(n * NCH, (n + 1) * NCH)
                if n == 0:
                    xin = xin0
                else:
                    xin = xpool.tile([P, KD, NCH], MM_DT, tag="xin")
                    nc.sync.dma_start(out=xin, in_=xT_r[:, :, lsl])

                for e in range(NE):
                    esl = slice(e * P, (e + 1) * P)
                    pg = pp.tile([P, NCH], F32, tag="pg")
                    pc = pp.tile([P, NCH], F32, tag="pc")
                    for kd in range(KD):
                        nc.tensor.matmul(
                            pg,
                            lhsT=wg_sb[kd][:, esl],
                            rhs=xin[:, kd, :],
                            start=(kd == 0),
                            stop=(kd == KD - 1),
                        )
                    for kd in range(KD):
                        nc.tensor.matmul(
                            pc,
                            lhsT=wh_sb[kd][:, esl],
                            rhs=xin[:, kd, :],
                            start=(kd == 0),
                            stop=(kd == KD - 1),
                        )
                    g = gc.tile([P, NCH], F32, tag="g")
                    nc.scalar.activation(
                        out=g, in_=pg, func=ACT.Sigmoid, bias=bg_sb[:, e : e + 1]
                    )
                    c = gc.tile([P, NCH], F32, tag="c")
                    nc.scalar.activation(
                        out=c, in_=pc, func=ACT.Tanh, bias=bh_sb[:, e : e + 1]
                    )
                    d1 = gc.tile([P, NCH], F32, tag="d1")
                    nc.vector.scalar_tensor_tensor(
                        out=d1, in0=g, scalar=1.0, in1=c,
                        op0=ALU.subtract, op1=ALU.mult,
                    )
                    init = (
                        h0_sb[:, e : e + 1]
                        if n == 0
                        else prev_h[e][:, NCH - 1 : NCH]
                    )
                    h = hpool.tile([P, NCH], F32, tag=f"h{e}")
                    nc.vector.tensor_tensor_scan(
                        out=h, data0=g, data1=d1, initial=init,
                        op0=ALU.mult, op1=ALU.subtract,
                    )
                    prev_h[e] = h
                    nc.sync.dma_start(out=out_r[:, e, lsl], in_=h)

    nc.compile()
    return nc


_NC_CACHE = None


def _get_nc():
    global _NC_CACHE
    if _NC_CACHE is None:
        _NC_CACHE = build_nc()
    return _NC_CACHE


def kernel(x, hidden, Wg, bg, Wh, bh):
    x = np.ascontiguousarray(np.asarray(x, dtype=np.float32))
    hidden = np.ascontiguousarray(np.asarray(hidden, dtype=np.float32))
    Wg = np.asarray(Wg, dtype=np.float32)
    bg = np.ascontiguousarray(np.asarray(bg, dtype=np.float32))
    Wh = np.asarray(Wh, dtype=np.float32)
    bh = np.ascontiguousarray(np.asarray(bh, dtype=np.float32))

    nc = _get_nc()

    xT = np.ascontiguousarray(x.transpose(0, 2, 1))   # [B, D, L]
    WgT = np.ascontiguousarray(Wg.T)
    WhT = np.ascontiguousarray(Wh.T)

    in_maps = [
        {
            "xT": xT[b],
            "WgT": WgT,
            "WhT": WhT,
            "bg": bg,
            "bh": bh,
            "hidden": hidden[b],
        }
        for b in range(B)
    ]
    res = bass_utils.run_bass_kernel_spmd(nc, in_maps, core_ids=list(range(B)))
    out = np.stack([res.results[b]["outT"].T for b in range(B)])  # [B, L, D]
    return np.ascontiguousarray(out.astype(np.float32))
